# revision 9
# baseline (speedup 1.0000x reference)
"""AddRelativePositionalEmbedding Trainium2 kernel.

Per-core problem (B=8 sharded 1 batch-head per core):
  out[r, k1*64+k2] = attn[r, k1*64+k2] + rel_h[r, k1] + rel_w[r, k2]
  rel_h[(h,w), k1] = sum_c q[(h,w),c] * rel_pos_h[h-k1+63, c]
  rel_w[(h,w), k2] = sum_c q[(h,w),c] * rel_pos_w[w-k2+63, c]

Memory-bound: 64MB in + 64MB out per core.  TensorE does the tiny einsums
(paired matmuls, 128 output rows each), DVE does two broadcast-adds per
streamed 128x4096 tile.  Aux DMAs ride the scalar (ACT) HWDGE ring, which
is idle early; SWDGE is avoided entirely -- its descriptor generation
arbitrates for the DVE/GpSimd shared SBUF port and slows every f32
tensor_tensor by ~20% for the rest of the run.
"""

import sys

if "/opt/trn_rl_repo" not in sys.path:
    sys.path.insert(0, "/opt/trn_rl_repo")

import numpy as np

import concourse.bass as bass
import concourse.tile as tile
from concourse import bacc, mybir
from concourse.bass import AP
from concourse.bass_utils import run_bass_kernel_spmd
from concourse.masks import make_identity

F32 = mybir.dt.float32
N_CORES = 8
QH = QW = KH = KW = 64
C = 64
NQ = QH * QW          # 4096 query positions per core
NK = KH * KW          # 4096 key positions
P = 128               # partitions per tile
NCHUNK = NQ // P      # 32 chunks of 128 query rows


def _ap(base: AP, extra_offset: int, dims: list[list[int]]) -> AP:
    """Build a raw AP on base's tensor at base.offset + extra_offset."""
    return AP(base.tensor, base.offset + extra_offset, [list(d) for d in dims])


def build_kernel_body(tc, attn_d: AP, q_d: AP, rph_d: AP, rpw_d: AP, out_d: AP):
    nc = tc.nc
    import contextlib

    ctx = contextlib.ExitStack()
    with ctx:
        consts = ctx.enter_context(tc.tile_pool(name="consts", bufs=1))
        ps_t = ctx.enter_context(tc.tile_pool(name="ps_t", bufs=2, space="PSUM"))
        ps_mm = ctx.enter_context(tc.tile_pool(name="ps_mm", bufs=4, space="PSUM"))
        stream = ctx.enter_context(tc.tile_pool(name="stream", bufs=4))

        # ---------------- Phase A: rel_h / rel_w (tiny einsums) ------------
        ident = consts.tile([P, P], F32)
        make_identity(nc, ident[:])

        # natural-layout loads (SWDGE so the HWDGE rings stay clear)
        rph_nat = consts.tile([2 * QH - 1, C], F32)
        nc.scalar.dma_start(rph_nat[:], rph_d)
        rpw_nat = consts.tile([2 * QW - 1, C], F32)
        nc.scalar.dma_start(rpw_nat[:], rpw_d)
        # q natural layout: q_nat[p, i*64 + c] = q[i*128 + p, c]
        q_nat = consts.tile([P, NCHUNK * C], F32)
        nc.scalar.dma_start(
            q_nat[:].rearrange("p (i c) -> p i c", c=C),
            q_d.rearrange("(i p) c -> p i c", p=P),
        )

        # transposed tables rphT/rpwT[c, idx] via PE transpose
        D = 2 * QH - 1
        rphT = consts.tile([C, D], F32)
        rpwT = consts.tile([C, D], F32)
        for src, dst in ((rph_nat, rphT), (rpw_nat, rpwT)):
            ps = ps_t.tile([C, P], F32, tag="ps_t")
            nc.tensor.transpose(ps[:, 0:D], src[:], ident[0:D, 0:D])
            nc.vector.tensor_copy(out=dst[:], in_=ps[:, 0:D])

        rphT_b = rphT[:]
        rpwT_b = rpwT[:]
        tp = rphT_b.ap[0][0]  # table partition pitch (elements)

        # qT[c, r] via PE transpose of each [128, 64] chunk of q_nat,
        # interleaved with the rel_h pair-matmuls that consume them.
        qT = consts.tile([C, NQ], F32)
        qT_b = qT[:]
        qp = qT_b.ap[0][0]

        # rel_h_sb[(h%2)*64 + w, (h//2)*64 + k1] -- chunk-row layout, built
        # directly by paired matmuls (h pair per 128-row chunk).
        rel_h_sb = consts.tile([P, NCHUNK * KH], F32)
        # stagingW2[h, w*64 + k2] from per-w rel_w matmuls
        stagingW2 = consts.tile([QH, QW * KW], F32)

        for i in range(NCHUNK):
            ps = ps_t.tile([C, P], F32, tag="ps_t")
            nc.tensor.transpose(ps[:], q_nat[:, i * C:(i + 1) * C], ident[:])
            nc.vector.tensor_copy(out=qT[:, i * P:(i + 1) * P], in_=ps[:])

            # rel_h pair-matmul for chunk i (h = 2i, 2i+1):
            # out[hh*64+w, hh'*64+k1] = sum_c qT[c, i*128 + hh*64 + w]
            #                               * rel_pos_hT[c, 2i+hh'+63-k1]
            pm = ps_mm.tile([P, P], F32, tag="ps_mm")
            rhs = _ap(rphT_b, 2 * i + KH - 1, [[tp, C], [1, 2], [-1, KH]])
            nc.tensor.matmul(
                pm[:].rearrange("p (a b) -> p a b", b=KH),
                qT_b[:, i * P:(i + 1) * P], rhs, start=True, stop=True,
            )
            # useful quadrants: rows 0:64 need hh'=0, rows 64:128 need hh'=1
            nc.vector.tensor_copy(out=rel_h_sb[0:64, i * KH:(i + 1) * KH],
                                  in_=pm[0:64, 0:KH])
            nc.vector.tensor_copy(out=rel_h_sb[64:P, i * KH:(i + 1) * KH],
                                  in_=pm[64:P, KH:2 * KH])

        # rel_w matmuls, one per w:
        # out[h, k2] = sum_c qT[c, h*64+w] * rel_pos_wT[c, w+63-k2]
        for w in range(QW):
            pm = ps_mm.tile([QH, KW], F32, tag="ps_mm")
            lhsT = _ap(qT_b, w, [[qp, C], [QW, QH]])
            rhs = _ap(rpwT_b, w + KW - 1, [[tp, C], [-1, KW]])
            nc.tensor.matmul(pm[:], lhsT, rhs, start=True, stop=True)
            nc.vector.tensor_copy(out=stagingW2[0:QH, w * KW:(w + 1) * KW],
                                  in_=pm[:])

        # rel_w needs a partition<->free shuffle (stagingW2 partition h maps
        # to dst free, staging free w maps to dst partition); SBUF APs can't
        # cross partitions in a non-leading dim, so bounce through DRAM where
        # APs are purely linear, then scatter with contiguous partition
        # blocks per DMA (h1 split) so writes to rel_w_sb don't interleave.
        rel_w_sb = consts.tile([P, NCHUNK * KW], F32)
        rw = rel_w_sb[:]
        rwp = rw.ap[0][0]
        scratch = nc.dram_tensor("scratch_w", [QH, QW * KW], F32)
        nc.scalar.dma_start(scratch.ap(), stagingW2[0:QH, :])
        SW = QW * KW  # scratch row pitch
        for h1 in range(2):
            # dst partition h1*64 + w, free h2*64 + k2
            # src scratch[h1 + 2*h2, w*64 + k2]
            nc.scalar.dma_start(
                _ap(rw, h1 * 64 * rwp, [[rwp, QW], [KW, NCHUNK], [1, KW]]),
                _ap(scratch.ap(), h1 * SW,
                    [[KW, QW], [2 * SW, NCHUNK], [1, KW]]),
            )

        rh = rel_h_sb[:]
        rhp = rh.ap[0][0]

        # ---------------- Phase B: stream the attention map ----------------
        for i in range(NCHUNK):
            t = stream.tile([P, NK], F32, tag="attn")
            nc.sync.dma_start(t[:], attn_d[i * P:(i + 1) * P, :])
            t3 = t[:].rearrange("p (a b) -> p a b", b=KW)
            relh = _ap(rh, i * KH, [[rhp, P], [1, KH], [0, KW]])
            relw = _ap(rw, i * KW, [[rwp, P], [0, KH], [1, KW]])
            nc.vector.tensor_tensor(out=t3, in0=t3, in1=relw, op=mybir.AluOpType.add)
            nc.vector.tensor_tensor(out=t3, in0=t3, in1=relh, op=mybir.AluOpType.add)
            nc.scalar.dma_start(out_d[i * P:(i + 1) * P, :], t[:])


_NC_CACHE = {}


def build_nc():
    if "nc" in _NC_CACHE:
        return _NC_CACHE["nc"]
    nc = bacc.Bacc("TRN2", target_bir_lowering=False, debug=False,
                   num_devices=N_CORES)
    attn = nc.dram_tensor("attention_map", [NQ, NK], F32, kind="ExternalInput")
    q = nc.dram_tensor("queries", [NQ, C], F32, kind="ExternalInput")
    rph = nc.dram_tensor("rel_pos_h", [2 * QH - 1, C], F32, kind="ExternalInput")
    rpw = nc.dram_tensor("rel_pos_w", [2 * QW - 1, C], F32, kind="ExternalInput")
    out = nc.dram_tensor("out", [NQ, NK], F32, kind="ExternalOutput")
    with tile.TileContext(nc) as tc:
        build_kernel_body(tc, attn.ap(), q.ap(), rph.ap(), rpw.ap(), out.ap())
    nc.compile()
    _NC_CACHE["nc"] = nc
    return nc


def make_in_maps(attention_map, queries, rel_pos_h, rel_pos_w):
    attn = np.ascontiguousarray(np.asarray(attention_map, dtype=np.float32))
    q = np.ascontiguousarray(np.asarray(queries, dtype=np.float32))
    rph = np.ascontiguousarray(np.asarray(rel_pos_h, dtype=np.float32))
    rpw = np.ascontiguousarray(np.asarray(rel_pos_w, dtype=np.float32))
    return [
        {"attention_map": attn[i], "queries": q[i],
         "rel_pos_h": rph, "rel_pos_w": rpw}
        for i in range(N_CORES)
    ]


def kernel(attention_map, queries, rel_pos_h, rel_pos_w,
           query_h=64, query_w=64, key_h=64, key_w=64, **_unused):
    nc = build_nc()
    in_maps = make_in_maps(attention_map, queries, rel_pos_h, rel_pos_w)
    res = run_bass_kernel_spmd(nc, in_maps, core_ids=list(range(N_CORES)))
    out = np.stack([res.results[i]["out"] for i in range(N_CORES)], axis=0)
    return out


# revision 12
# speedup vs baseline: 1.1917x; 1.1917x over previous
"""AddRelativePositionalEmbedding Trainium2 kernel.

Per-core problem (B=8 sharded 1 batch-head per core):
  out[r, k1*64+k2] = attn[r, k1*64+k2] + rel_h[r, k1] + rel_w[r, k2]
  rel_h[(h,w), k1] = sum_c q[(h,w),c] * rel_pos_h[h-k1+63, c]
  rel_w[(h,w), k2] = sum_c q[(h,w),c] * rel_pos_w[w-k2+63, c]

Memory-bound: 64MB in + 64MB out per core.  TensorE does the tiny einsums
(paired matmuls, 128 output rows each), DVE does two broadcast-adds per
streamed 128x4096 tile.  Aux DMAs ride the scalar (ACT) HWDGE ring, which
is idle early; SWDGE is avoided entirely -- its descriptor generation
arbitrates for the DVE/GpSimd shared SBUF port and slows every f32
tensor_tensor by ~20% for the rest of the run.
"""

import sys

if "/opt/trn_rl_repo" not in sys.path:
    sys.path.insert(0, "/opt/trn_rl_repo")

import numpy as np

import concourse.bass as bass
import concourse.tile as tile
from concourse import bacc, mybir
from concourse.bass import AP
from concourse.bass_utils import run_bass_kernel_spmd
from concourse.masks import make_identity

F32 = mybir.dt.float32
N_CORES = 8
QH = QW = KH = KW = 64
C = 64
NQ = QH * QW          # 4096 query positions per core
NK = KH * KW          # 4096 key positions
P = 128               # partitions per tile
NCHUNK = NQ // P      # 32 chunks of 128 query rows


def _ap(base: AP, extra_offset: int, dims: list[list[int]]) -> AP:
    """Build a raw AP on base's tensor at base.offset + extra_offset."""
    return AP(base.tensor, base.offset + extra_offset, [list(d) for d in dims])


def build_kernel_body(tc, attn_d: AP, q_d: AP, rph_d: AP, rpw_d: AP, out_d: AP):
    nc = tc.nc
    import contextlib

    ctx = contextlib.ExitStack()
    with ctx:
        consts = ctx.enter_context(tc.tile_pool(name="consts", bufs=1))
        ps_t = ctx.enter_context(tc.tile_pool(name="ps_t", bufs=2, space="PSUM"))
        ps_mm = ctx.enter_context(tc.tile_pool(name="ps_mm", bufs=4, space="PSUM"))
        stream = ctx.enter_context(tc.tile_pool(name="stream", bufs=6))

        # ---------------- Phase A: rel_h / rel_w (tiny einsums) ------------
        ident = consts.tile([P, P], F32)
        make_identity(nc, ident[:])

        # Aux loads go FIRST on the sync ring (descriptor-light), ahead of the
        # attention stream; tiny-descriptor DMAs on the out (ACT) ring starve
        # behind streaming packets and complete ~25us late.
        rph_nat = consts.tile([2 * QH - 1, C], F32)
        nc.sync.dma_start(rph_nat[:], rph_d)
        rpw_nat = consts.tile([2 * QW - 1, C], F32)
        nc.sync.dma_start(rpw_nat[:], rpw_d)
        # q in partition-block layout: q_lin[p, j*64 + c] = q[p*32 + j, c]
        # (128 descriptors x 8KB, fully contiguous per partition)
        q_lin = consts.tile([P, NCHUNK * C], F32)
        nc.sync.dma_start(
            q_lin[:].rearrange("p (j c) -> p j c", c=C),
            q_d.rearrange("(p j) c -> p j c", p=P),
        )

        # transposed tables rphT/rpwT[c, idx] via PE transpose
        D = 2 * QH - 1
        rphT = consts.tile([C, D], F32)
        rpwT = consts.tile([C, D], F32)
        for src, dst in ((rph_nat, rphT), (rpw_nat, rpwT)):
            ps = ps_t.tile([C, P], F32, tag="ps_t")
            nc.tensor.transpose(ps[:, 0:D], src[:], ident[0:D, 0:D])
            nc.vector.tensor_copy(out=dst[:], in_=ps[:, 0:D])

        rphT_b = rphT[:]
        rpwT_b = rpwT[:]
        tp = rphT_b.ap[0][0]  # table partition pitch (elements)

        # qT[c, r] via PE transpose of each [128, 64] slice of q_lin.
        # Slice j holds rows {p*32 + j : p in [0,128)} so the psum result is
        # copied to qT with a stride-32 free pattern.
        qT = consts.tile([C, NQ], F32)
        qT_b = qT[:]
        qp = qT_b.ap[0][0]
        for j in range(NCHUNK):
            ps = ps_t.tile([C, P], F32, tag="ps_t")
            nc.tensor.transpose(ps[:], q_lin[:, j * C:(j + 1) * C], ident[:])
            nc.vector.tensor_copy(
                out=_ap(qT_b, j, [[qp, C], [NCHUNK, P]]), in_=ps[:])

        # rel_w first: it feeds the first add of every streamed chunk (the
        # bounce+scatter sits on its critical path), rel_h matmuls can lag.
        # rel_w matmuls, one per w:
        # out[h, k2] = sum_c qT[c, h*64+w] * rel_pos_wT[c, w+63-k2]
        stagingW2 = consts.tile([QH, QW * KW], F32)
        for w in range(QW):
            pm = ps_mm.tile([QH, KW], F32, tag="ps_mm")
            lhsT = _ap(qT_b, w, [[qp, C], [QW, QH]])
            rhs = _ap(rpwT_b, w + KW - 1, [[tp, C], [-1, KW]])
            nc.tensor.matmul(pm[:], lhsT, rhs, start=True, stop=True)
            nc.vector.tensor_copy(out=stagingW2[0:QH, w * KW:(w + 1) * KW],
                                  in_=pm[:])

        # rel_h_sb[(h%2)*64 + w, (h//2)*64 + k1] -- chunk-row layout, built
        # directly by paired matmuls (h pair per 128-row chunk):
        # out[hh*64+w, hh'*64+k1] = sum_c qT[c, i*128 + hh*64 + w]
        #                               * rel_pos_hT[c, 2i+hh'+63-k1]
        rel_h_sb = consts.tile([P, NCHUNK * KH], F32)
        for i in range(NCHUNK):
            pm = ps_mm.tile([P, P], F32, tag="ps_mm")
            rhs = _ap(rphT_b, 2 * i + KH - 1, [[tp, C], [1, 2], [-1, KH]])
            nc.tensor.matmul(
                pm[:].rearrange("p (a b) -> p a b", b=KH),
                qT_b[:, i * P:(i + 1) * P], rhs, start=True, stop=True,
            )
            # useful quadrants: rows 0:64 need hh'=0, rows 64:128 need hh'=1
            nc.vector.tensor_copy(out=rel_h_sb[0:64, i * KH:(i + 1) * KH],
                                  in_=pm[0:64, 0:KH])
            nc.vector.tensor_copy(out=rel_h_sb[64:P, i * KH:(i + 1) * KH],
                                  in_=pm[64:P, KH:2 * KH])

        # rel_w needs a partition<->free shuffle (stagingW2 partition h maps
        # to dst free, staging free w maps to dst partition); SBUF APs can't
        # cross partitions in a non-leading dim, so bounce through DRAM where
        # APs are purely linear, then scatter with contiguous partition
        # blocks per DMA (h1 split) so writes to rel_w_sb don't interleave.
        rel_w_sb = consts.tile([P, NCHUNK * KW], F32)
        rw = rel_w_sb[:]
        rwp = rw.ap[0][0]
        scratch = nc.dram_tensor("scratch_w", [QH, QW * KW], F32)
        nc.scalar.dma_start(scratch.ap(), stagingW2[0:QH, :])
        SW = QW * KW  # scratch row pitch
        for h1 in range(2):
            # dst partition h1*64 + w, free h2*64 + k2
            # src scratch[h1 + 2*h2, w*64 + k2]
            nc.scalar.dma_start(
                _ap(rw, h1 * 64 * rwp, [[rwp, QW], [KW, NCHUNK], [1, KW]]),
                _ap(scratch.ap(), h1 * SW,
                    [[KW, QW], [2 * SW, NCHUNK], [1, KW]]),
            )

        rh = rel_h_sb[:]
        rhp = rh.ap[0][0]

        # ---------------- Phase B: stream the attention map ----------------
        for i in range(NCHUNK):
            t = stream.tile([P, NK], F32, tag="attn")
            nc.sync.dma_start(t[:], attn_d[i * P:(i + 1) * P, :])
            t3 = t[:].rearrange("p (a b) -> p a b", b=KW)
            relh = _ap(rh, i * KH, [[rhp, P], [1, KH], [0, KW]])
            relw = _ap(rw, i * KW, [[rwp, P], [0, KH], [1, KW]])
            nc.vector.tensor_tensor(out=t3, in0=t3, in1=relw, op=mybir.AluOpType.add)
            nc.vector.tensor_tensor(out=t3, in0=t3, in1=relh, op=mybir.AluOpType.add)
            nc.scalar.dma_start(out_d[i * P:(i + 1) * P, :], t[:])


_NC_CACHE = {}


def build_nc():
    if "nc" in _NC_CACHE:
        return _NC_CACHE["nc"]
    nc = bacc.Bacc("TRN2", target_bir_lowering=False, debug=False,
                   num_devices=N_CORES)
    attn = nc.dram_tensor("attention_map", [NQ, NK], F32, kind="ExternalInput")
    q = nc.dram_tensor("queries", [NQ, C], F32, kind="ExternalInput")
    rph = nc.dram_tensor("rel_pos_h", [2 * QH - 1, C], F32, kind="ExternalInput")
    rpw = nc.dram_tensor("rel_pos_w", [2 * QW - 1, C], F32, kind="ExternalInput")
    out = nc.dram_tensor("out", [NQ, NK], F32, kind="ExternalOutput")
    with tile.TileContext(nc) as tc:
        build_kernel_body(tc, attn.ap(), q.ap(), rph.ap(), rpw.ap(), out.ap())
    nc.compile()
    _NC_CACHE["nc"] = nc
    return nc


def make_in_maps(attention_map, queries, rel_pos_h, rel_pos_w):
    attn = np.ascontiguousarray(np.asarray(attention_map, dtype=np.float32))
    q = np.ascontiguousarray(np.asarray(queries, dtype=np.float32))
    rph = np.ascontiguousarray(np.asarray(rel_pos_h, dtype=np.float32))
    rpw = np.ascontiguousarray(np.asarray(rel_pos_w, dtype=np.float32))
    return [
        {"attention_map": attn[i], "queries": q[i],
         "rel_pos_h": rph, "rel_pos_w": rpw}
        for i in range(N_CORES)
    ]


def kernel(attention_map, queries, rel_pos_h, rel_pos_w,
           query_h=64, query_w=64, key_h=64, key_w=64, **_unused):
    nc = build_nc()
    in_maps = make_in_maps(attention_map, queries, rel_pos_h, rel_pos_w)
    res = run_bass_kernel_spmd(nc, in_maps, core_ids=list(range(N_CORES)))
    out = np.stack([res.results[i]["out"] for i in range(N_CORES)], axis=0)
    return out
